# revision 2
# baseline (speedup 1.0000x reference)
"""Trainium2 Bass kernel v2 for KroneckerLinear: out = x @ kron(f1,f2).T + bias.

bf16 I/O redesign (memory-roofline target):
  - Host casts x to bf16 and views the per-core shard as x2 [R2, 128]
    (pairs of rows), so the 64x64 weight becomes the 128x128
    block-diagonal W2 = diag(wT, wT) and matmuls contract the full 128.
  - Input tiles are loaded with the XBAR DMA transpose (HWDGE) directly
    into [feature=128, rows] layout - no TensorE transpose, no
    PSUM round-trip on the input side.
  - Per 256-row chunk, two self-loading bf16 matmuls with the
    transposed chunk as the stationary operand (even rows / odd rows via
    stride-2 APs) and W2 as the moving operand write a [128, 256] PSUM
    block whose partition p holds rows 2p,2p+1 => the output DMA sees
    512 B contiguous DRAM segments (HBM line rate).
  - DVE/Pool alternate on PSUM->SBUF bias-add copies (bf16 out).
  - Output returns as bf16; host upcasts to f32.
Device traffic: 32 MiB in + 32 MiB out per core ~= 188 us at 358 GB/s.
"""

import numpy as np
import ml_dtypes
from contextlib import ExitStack

from concourse import bacc, bass, mybir, tile
from concourse.bass_utils import run_bass_kernel_spmd

N_CORES = 8
N_ROWS = 2097152
D = 64

R = N_ROWS // N_CORES  # rows per core = 262144
R2 = R // 2  # packed rows per core = 131072 (x2 is [R2, 128])
F2 = 128
TILE = 4096  # x2 rows per DMA tile (1 MiB bf16)
CH = 2048  # x2 rows per psum chunk (16 matmuls, [128, 2048] f32 = 4 banks)
GROUP = 1  # chunks per psum tile / copy instruction

FP = mybir.dt.float32
BF = mybir.dt.bfloat16
BF_NP = ml_dtypes.bfloat16

_CACHE = {}


# Shape-salt: the HLO module hash does NOT cover the embedded bass
# payload, only tensor shapes — two different kernels with identical
# I/O shapes collide in the NEFF disk cache. A per-version dummy input
# whose SHAPE encodes the kernel version keeps the hash unique.
KERNEL_VERSION = 203


def _build_nc(r2=R2, fori=None, variant="full", tile_rows=TILE,
              in_bufs=4, out_bufs=4, psum_bufs=2, copy_engines="v",
              ch=CH, group=GROUP, batch=1, out_dma="s",
              salt=KERNEL_VERSION):
    nc = bacc.Bacc("TRN2", target_bir_lowering=False, debug=False)

    stride = ch // 128  # rows per psum partition = matmuls per chunk
    n_tiles = r2 // tile_rows
    chunks = tile_rows // ch  # psum chunks per tile
    groups = chunks // group  # copy instructions per tile

    x2 = nc.dram_tensor("x2", [r2, F2], BF, kind="ExternalInput")
    w2 = nc.dram_tensor("w2", [128, 128], BF, kind="ExternalInput")
    bias_big = nc.dram_tensor("bias_big", [128, group * ch], FP,
                              kind="ExternalInput")
    nc.dram_tensor("salt", [1, salt], FP, kind="ExternalInput")
    out2 = nc.dram_tensor("out2", [r2, F2], BF, kind="ExternalOutput")

    with ExitStack() as ctx:
        tc = ctx.enter_context(tile.TileContext(nc))

        consts = ctx.enter_context(tc.tile_pool(name="consts", bufs=1))
        w2_sb = consts.tile([128, 128], BF)
        nc.sync.dma_start(w2_sb[:], w2[:, :])
        bias_sb = consts.tile([128, group * ch], FP)
        nc.sync.dma_start(bias_sb[:], bias_big[:, :])

        in_pool = ctx.enter_context(tc.tile_pool(name="in_pool",
                                                 bufs=in_bufs))
        out_psum = ctx.enter_context(
            tc.tile_pool(name="out_psum", bufs=psum_bufs, space="PSUM"))
        out_pool = ctx.enter_context(tc.tile_pool(name="out_pool",
                                                  bufs=out_bufs))

        loop_ctx = tc.For_i(0, fori, 1) if fori is not None else None
        if loop_ctx is not None:
            loop_ctx.__enter__()

        noio_in = None
        if variant in ("noio", "pe_only", "cp_only", "out_only"):
            noio_in = in_pool.tile([128, tile_rows], BF)
            nc.sync.dma_start_transpose(noio_in[:], x2[0:tile_rows, :])

        eng_map = {"v": nc.vector, "g": nc.gpsimd, "s": nc.scalar}
        copy_engs = [eng_map[c] for c in copy_engines]
        out_eng = {"s": nc.scalar, "g": nc.gpsimd, "y": nc.sync}[out_dma]

        def out_ap_for(r0):
            # DRAM out view: row (l p s) = r0 + l*ch + p*stride + s;
            # per partition: l segments of (s f)=ch elems = 2*ch bytes.
            return out2[r0:r0 + tile_rows, :].rearrange(
                "(l p s) f -> p l (s f)", p=128, s=stride)

        for tb in range(0, n_tiles, batch):
            tbatch = range(tb, min(tb + batch, n_tiles))
            in_ts = {}
            for t in tbatch:
                r0 = t * tile_rows
                if variant in ("noio", "pe_only", "cp_only", "out_only"):
                    in_ts[t] = noio_in
                elif variant in ("dmaonly_nat", "in_only_nat"):
                    # natural-layout load (no XBAR): partition p holds
                    # tile_rows/128 consecutive x2 rows.
                    in_ts[t] = in_pool.tile([128, tile_rows], BF,
                                            name="in_t")
                    nc.sync.dma_start(
                        in_ts[t][:],
                        x2[r0:r0 + tile_rows, :].rearrange(
                            "(p l) f -> p (l f)", p=128))
                else:
                    in_ts[t] = in_pool.tile([128, tile_rows], BF,
                                            name="in_t")
                    nc.sync.dma_start_transpose(
                        in_ts[t][:], x2[r0:r0 + tile_rows, :])

            if variant in ("in_only", "in_only_nat"):
                if n_tiles - 1 in tbatch:
                    out_eng.dma_start(
                        out_ap_for((n_tiles - 1) * tile_rows),
                        in_ts[n_tiles - 1][:].rearrange(
                            "p (l c) -> p l c", c=ch))
                continue
            if variant in ("dmaonly", "dmaonly_nat", "out_only"):
                for t in tbatch:
                    out_eng.dma_start(
                        out_ap_for(t * tile_rows),
                        in_ts[t][:].rearrange("p (l c) -> p l c", c=ch))
                continue

            out_ts = {}
            for t in tbatch:
                in_t = in_ts[t]
                out_t = out_ts[t] = out_pool.tile([128, tile_rows], BF,
                                                  name="out_t")
                for g in range(groups):
                    op = out_psum.tile([128, group * ch], FP)
                    for j in range(group):
                        c = g * group + j
                        # xT chunk columns (m s): column m*stride+s = chunk
                        # row m*stride+s. Matmul si takes the stride-spaced
                        # columns {m*stride+si} so psum partition m holds
                        # rows m*stride..m*stride+stride-1 -> contiguous
                        # DRAM segments per partition.
                        xt = in_t[:, c * ch:(c + 1) * ch].rearrange(
                            "k (m s) -> k s m", s=stride)
                        for si in range(stride):
                            nc.tensor.matmul(
                                op[:, j * ch + si * 128:
                                   j * ch + (si + 1) * 128],
                                xt[:, si, :], w2_sb[:],
                                start=True, stop=True)
                    if variant == "pe_only":
                        continue
                    eng = copy_engs[g % len(copy_engs)]
                    eng.tensor_add(
                        out_t[:, g * ch * group:(g + 1) * ch * group],
                        op[:], bias_sb[:])

            for t in tbatch:
                if variant == "full" or (variant in ("noio", "cp_only")
                                         and t == n_tiles - 1):
                    out_eng.dma_start(
                        out_ap_for(t * tile_rows),
                        out_ts[t][:].rearrange("p (l c) -> p l c", c=ch))

        if loop_ctx is not None:
            loop_ctx.__exit__(None, None, None)

    nc.compile()
    return nc


def _get_nc():
    if "nc" not in _CACHE:
        _CACHE["nc"] = _build_nc()
    return _CACHE["nc"]


def _make_bias_big(bias, ch=CH, group=GROUP):
    b2 = np.concatenate([np.asarray(bias, np.float32)] * 2)  # [128]
    blk = np.tile(b2, (ch // 128) * group)  # (s f) x group
    return np.ascontiguousarray(
        np.broadcast_to(blk, (128, group * ch)).astype(np.float32))


def _prep_in_maps(x, factor1, factor2, bias):
    w = np.kron(np.asarray(factor1, np.float32),
                np.asarray(factor2, np.float32))  # [64, 64]
    w2 = np.zeros((128, 128), dtype=np.float32)
    w2[:64, :64] = w.T
    w2[64:, 64:] = w.T
    w2b = w2.astype(BF_NP)
    bias_big = _make_bias_big(bias)

    xb = np.asarray(x, dtype=np.float32).astype(BF_NP)
    x2 = np.ascontiguousarray(xb).reshape(N_ROWS // 2, F2)
    salt = np.zeros((1, KERNEL_VERSION), np.float32)
    in_maps = []
    for c in range(N_CORES):
        in_maps.append({
            "x2": x2[c * R2:(c + 1) * R2],
            "w2": w2b,
            "bias_big": bias_big,
            "salt": salt,
        })
    return in_maps


def run(inputs, trace=False, **run_kwargs):
    nc = _get_nc()
    in_maps = _prep_in_maps(**inputs)
    try:
        res = run_bass_kernel_spmd(nc, in_maps, list(range(N_CORES)),
                                   trace=trace, **run_kwargs)
    except Exception:
        res = run_bass_kernel_spmd(nc, in_maps, list(range(N_CORES)),
                                   trace=trace, **run_kwargs)
    shards = [np.asarray(res.results[c]["out2"]) for c in range(N_CORES)]
    out = np.concatenate(shards, axis=0).astype(np.float32)
    return out.reshape(N_ROWS, D), res


def kernel(x, factor1, factor2, bias):
    out, _ = run(dict(x=x, factor1=factor1, factor2=factor2, bias=bias))
    return out


if __name__ == "__main__":
    # Small-shape numeric check against numpy on hardware.
    rng = np.random.default_rng(1)
    r2 = 16384
    x2 = rng.standard_normal((r2, 128)).astype(np.float32)
    f1 = rng.standard_normal((8, 8)).astype(np.float32)
    f2 = rng.standard_normal((8, 8)).astype(np.float32)
    bias = rng.standard_normal(64).astype(np.float32)
    w = np.kron(f1, f2)
    w2 = np.zeros((128, 128), np.float32)
    w2[:64, :64] = w.T
    w2[64:, 64:] = w.T

    nc = _build_nc(r2=r2)
    im = {
        "x2": x2.astype(BF_NP),
        "w2": w2.astype(BF_NP),
        "bias_big": _make_bias_big(bias),
        "salt": np.zeros((1, KERNEL_VERSION), np.float32),
    }
    res = run_bass_kernel_spmd(nc, [im] * N_CORES, list(range(N_CORES)))
    got = np.asarray(res.results[0]["out2"]).astype(np.float32)
    b2 = np.concatenate([bias, bias])
    want = x2 @ w2 + b2
    err = np.abs(got - want)
    scale = np.abs(want).max()
    print(f"absmax {err.max():.4e} scale {scale:.4e} rel {err.max()/scale:.3e}")
    denom = np.maximum(np.abs(want), 1e-3 * scale)
    print(f"elemwise rel max {(err/denom).max():.3e}")
    for c in range(1, N_CORES):
        gc = np.asarray(res.results[c]["out2"]).astype(np.float32)
        assert np.array_equal(gc, got.astype(BF_NP).astype(np.float32)), c
    print("all cores identical: OK")
